# revision 20
# baseline (speedup 1.0000x reference)
"""Causal self-attention (B=4, T=2048, C=1024, NH=16) on 8 trn2 NeuronCores.

Sharding: core = (batch b, head-half g); each core computes 8 heads of one
batch element and a partial projection output; host sums the two partials
per batch and folds in b_proj and the (softmax-row-sum==1) v-bias term.

b_attn's q/k components are assumed zero (spec fill: "zeros").

All matmul inputs are bf16 (1 cycle/row on the PE, cheap fast-weight-load
LDWEIGHTS, half the HBM traffic); accumulation is always f32 in PSUM.

Pipeline design (the previous version lost 2x+ to the PE HAM clock gate:
ScalarE exp was the per-head rate limiter, the PE micro-idled waiting on
it, and HAM throttled the PE clock to K=4/8 for ~380us of the attention
phase):
 - S^T matmuls write 2-key-chunk [128, 2, 512] PSUM tiles; ONE batched
   exp per tile ([128,1024]) amortizes ScalarE's 352-cycle fixed cost.
 - Heads are staggered: the PE stream interleaves S(h) tiles with
   PV(h-1) chunks, so the PE always has ready work while ScalarE exps
   head h (PV(h-1) inputs were finished last block).
 - Softmax normalization: DVE reciprocal_approx_fast on the ones-column
   rowsum (no Ln/Exp round trip, no activation-table switches), then a
   PE outer-product broadcast and one DVE multiply straight out of PSUM.
 - Projection of span s-1 is interleaved into the attention of span s,
   and projection results DMA to DRAM directly from PSUM.
"""

from contextlib import ExitStack

import ml_dtypes
import numpy as np

import concourse.bass as bass  # noqa: F401
import concourse.mybir as mybir
import concourse.tile as tile
from concourse import bacc
from concourse.bass_utils import run_bass_kernel_spmd

B, T, C, NH = 4, 2048, 1024, 16
HD = 64
NCORES = 8
HPC = NH // 2            # heads per core
DH = HPC * HD            # 512 per-core qkv feature width
TS = T // 512            # 4 query spans of 512
NT = T // 128            # 16 tiles of 128
NC_CHUNKS = C // 128     # 8 contraction chunks

F32 = mybir.dt.float32
F32R = mybir.dt.float32r
BF16 = mybir.dt.bfloat16
EXP = mybir.ActivationFunctionType.Exp

TRACE = False            # set by test.py for profiled runs
TRACE_KW = {}
LAST_RESULT = None

_nc_cache = None


def _build():
    nc = bacc.Bacc("TRN2", target_bir_lowering=False)

    xT_d = nc.dram_tensor("xT", [C, T], BF16, kind="ExternalInput")
    wqk_d = nc.dram_tensor("wqk", [8, NC_CHUNKS, 128, 128], BF16, kind="ExternalInput")
    wv_d = nc.dram_tensor("wv", [C, DH], BF16, kind="ExternalInput")
    wp_d = nc.dram_tensor("wp", [DH, C], BF16, kind="ExternalInput")
    maskB_d = nc.dram_tensor("maskB", [128, 4, 128], BF16, kind="ExternalInput")
    vones_d = nc.dram_tensor("vones", [128, HPC], BF16, kind="ExternalInput")
    ones64_d = nc.dram_tensor("ones64", [1, 64], BF16, kind="ExternalInput")
    out_d = nc.dram_tensor("out", [T, C], F32, kind="ExternalOutput")

    with tile.TileContext(nc) as tc, ExitStack() as ctx:
        const = ctx.enter_context(tc.tile_pool(name="const", bufs=1))
        persist = ctx.enter_context(tc.tile_pool(name="persist", bufs=1))

        maskB = const.tile([128, 4, 128], BF16)
        nc.sync.dma_start(maskB[:], maskB_d[:])
        ones64 = const.tile([1, 64], BF16)
        nc.sync.dma_start(ones64[:], ones64_d[:])

        # persistent SBUF: qT/kT bf16 [feat, T] (chunks 0-3 q, 4-7 k),
        # V bf16 [T-tile, head, 64+ones-col], wp bf16
        qk_sb = [persist.tile([128, T], BF16, tag=f"qk{i}", name=f"qk{i}")
                 for i in range(8)]
        v_sb = [persist.tile([128, HPC, 65], BF16, tag=f"v{i}", name=f"v{i}")
                for i in range(NT)]
        wp_sb = [persist.tile([128, C], BF16, tag=f"wp{i}", name=f"wp{i}")
                 for i in range(DH // 128)]
        for c in range(DH // 128):
            nc.sync.dma_start(wp_sb[c][:], wp_d[c * 128:(c + 1) * 128, :])
        for t in range(NT):
            nc.sync.dma_start(v_sb[t][:, :, 64], vones_d[:])

        # ---- merged QKV-projection + attention + projection --------------
        with tc.tile_pool(name="xT", bufs=1) as xpool, \
             tc.tile_pool(name="wqk", bufs=1) as wqkpool, \
             tc.tile_pool(name="wv", bufs=1) as wvpool, \
             tc.tile_pool(name="pt", bufs=1) as ptpool, \
             tc.tile_pool(name="yts", bufs=1) as ytspool, \
             tc.tile_pool(name="small", bufs=2) as small, \
             tc.tile_pool(name="outsb", bufs=2) as outsb, \
             tc.tile_pool(name="stps", bufs=2, space="PSUM") as stps, \
             tc.tile_pool(name="otps", bufs=2, space="PSUM") as otps, \
             tc.tile_pool(name="pprb", bufs=2, space="PSUM") as pprb:

            # DMA order matters: the first qk matmuls need xT span 0 and
            # the wqk chunks, so queue those ahead of the bulk of xT.
            xT_sb = [xpool.tile([128, T], BF16, tag=f"x{c}", name=f"x{c}")
                     for c in range(NC_CHUNKS)]
            for c in range(NC_CHUNKS):
                nc.sync.dma_start(
                    xT_sb[c][:, 0:512], xT_d[c * 128:(c + 1) * 128, 0:512])
            wts = []
            for ft in range(8):
                row = []
                for c in range(NC_CHUNKS):
                    wt = wqkpool.tile([128, 128], BF16, tag=f"wqk{ft}_{c}",
                                      name=f"wqk{ft}_{c}")
                    nc.sync.dma_start(wt[:], wqk_d[ft, c])
                    row.append(wt)
                wts.append(row)
            wv_sb = [wvpool.tile([128, DH], BF16, tag=f"wv{c}", name=f"wv{c}")
                     for c in range(NC_CHUNKS)]
            for c in range(NC_CHUNKS):
                nc.sync.dma_start(wv_sb[c][:], wv_d[c * 128:(c + 1) * 128, :])
            for ts in range(1, TS):
                for c in range(NC_CHUNKS):
                    nc.sync.dma_start(
                        xT_sb[c][:, ts * 512:(ts + 1) * 512],
                        xT_d[c * 128:(c + 1) * 128, ts * 512:(ts + 1) * 512])

            def qk_tile(ts, ft):
                ps = pprb.tile([128, 512], F32, tag="pp", name="qkp")
                for c in range(NC_CHUNKS):
                    nc.tensor.matmul(
                        ps[:], wts[ft][c][:],
                        xT_sb[c][:, ts * 512:(ts + 1) * 512],
                        start=(c == 0), stop=(c == NC_CHUNKS - 1))
                nc.vector.tensor_copy(
                    qk_sb[ft][:, ts * 512:(ts + 1) * 512], ps[:])

            def v_tile(t):
                vp = pprb.tile([128, 512], F32, tag="pp", name="vp")
                for c in range(NC_CHUNKS):
                    nc.tensor.matmul(
                        vp[:], xT_sb[c][:, t * 128:(t + 1) * 128],
                        wv_sb[c][:],
                        start=(c == 0), stop=(c == NC_CHUNKS - 1))
                nc.vector.tensor_copy(
                    v_sb[t][:, :, 0:64],
                    vp.rearrange("p (h d) -> p h d", h=HPC))

            # P~^T scratch, double buffered across heads:
            # [k-part, j-chunk, q-span], bf16
            pt = [ptpool.tile([128, NT, 512], BF16, tag=f"pt{i}",
                              name=f"pt{i}") for i in range(2)]
            # normalized attention outputs, double buffered across spans
            yts = [[ytspool.tile([128, 512], BF16, tag=f"yts{p}_{i}",
                                 name=f"yts{p}_{i}")
                    for i in range(DH // 128)] for p in range(2)]

            def s_tile(s, h, jt):
                """Two S^T chunk matmuls + masks + one batched exp."""
                qch, qrow = h // 2, 64 * (h % 2)
                qT = qk_sb[qch]
                kT = qk_sb[4 + qch]
                st = stps.tile([128, 2, 512], F32, tag="st", name="st")
                js = (2 * jt, 2 * jt + 1)
                for sl, j in enumerate(js):
                    qo = max(s * 512, j * 128)
                    w = (s + 1) * 512 - qo
                    nc.tensor.matmul(
                        st[:, sl, :w],
                        kT[qrow:qrow + 64, j * 128:(j + 1) * 128],
                        qT[qrow:qrow + 64, qo:qo + w],
                        start=True, stop=True)
                nc.scalar.activation(
                    pt[h % 2][:, js[0]:js[0] + 2, :], st[:, :, :], EXP)

            def mask_head(s, h):
                # multiplicative 0/1 causal mask on the 4 diagonal chunks'
                # first 128 columns, applied to pt AFTER exp: keeps the
                # Vector op off the exp critical path (PV reads pt a full
                # head-block later).
                nc.vector.tensor_tensor(
                    pt[h % 2][:, 4 * s:4 * s + 4, 0:128],
                    pt[h % 2][:, 4 * s:4 * s + 4, 0:128],
                    maskB[:], mybir.AluOpType.mult)

            def pv_chunks(s, h, jt):
                """Two P@V chunk matmuls for head h (exp'd last block)."""
                jmax = 4 * s + 3
                for j in (2 * jt, 2 * jt + 1):
                    qo = max(s * 512, j * 128)
                    w = (s + 1) * 512 - qo
                    rel = qo - s * 512
                    if j == 0:
                        ot = otps.tile([128, 512], F32, tag="ot", name="ot")
                        pv_chunks.ot = ot
                    ot = pv_chunks.ot
                    nc.tensor.matmul(
                        ot[0:65, rel:rel + w],
                        v_sb[j][:, h, :], pt[h % 2][:, j, :w],
                        start=(j == 0), stop=(j == jmax),
                        skip_group_check=True)
                return pv_chunks.ot

            def norm(s, h, ot):
                """yts(head block) = ot[0:64] * broadcast(1/rowsum)."""
                qch, qrow = h // 2, 64 * (h % 2)
                rsum = small.tile([1, 512], F32, tag="rsum", name="rsum")
                nc.vector.tensor_copy(rsum[:], ot[64:65, :])
                rinv = small.tile([1, 512], F32, tag="rinv", name="rinv")
                nc.vector.reciprocal_approx_fast(out=rinv[:], in_=rsum[:])
                rinvb = small.tile([1, 512], BF16, tag="rinvb", name="rinvb")
                nc.vector.tensor_copy(rinvb[:], rinv[:])
                rb = pprb.tile([128, 512], F32, tag="pp", name="rb")
                nc.tensor.matmul(rb[0:64, :], ones64[:], rinvb[:],
                                 start=True, stop=True)
                rbs = small.tile([64, 512], F32, tag="rbs", name="rbs")
                nc.vector.tensor_copy(rbs[:], rb[0:64, :])
                nc.vector.tensor_tensor(
                    yts[s % 2][qch][qrow:qrow + 64, :], ot[0:64, :],
                    rbs[:], mybir.AluOpType.mult)

            def proj_t4(sp, t4):
                """Project one 128-query tile of span sp; DMA from PSUM."""
                tt = sp * 4 + t4
                for n in range(2):
                    po = pprb.tile([128, 512], F32, tag="pp", name="pp")
                    for c in range(DH // 128):
                        nc.tensor.matmul(
                            po[:],
                            yts[sp % 2][c][:, t4 * 128:(t4 + 1) * 128],
                            wp_sb[c][:, n * 512:(n + 1) * 512],
                            start=(c == 0), stop=(c == DH // 128 - 1))
                    ob = outsb.tile([128, 512], F32, tag="ob", name="ob")
                    nc.vector.tensor_copy(ob[:], po[:])
                    nc.sync.dma_start(
                        out_d[tt * 128:(tt + 1) * 128,
                              n * 512:(n + 1) * 512], ob[:])

            def filler(f):
                if f[0] == "qk":
                    qk_tile(f[1], f[2])
                else:
                    v_tile(f[1])

            # span-0 filler placement: everything a block consumes is
            # emitted in an earlier block (in-order PE queue => deadlock
            # otherwise). Head h needs qk(0,h//2)+qk(0,4+h//2); PV chunk
            # j needs V(j).
            s0_block = {
                0: [("qk", 0, 1), ("v", 0), ("qk", 0, 5), ("v", 1)],
                1: [("qk", 0, 2)], 2: [("qk", 0, 6)],
                3: [("qk", 0, 3), ("qk", 1, 0)],
                4: [("qk", 0, 7), ("qk", 1, 1)],
                5: [("qk", 1, 2), ("qk", 1, 4)],
                6: [("qk", 1, 5), ("v", 4)],
                7: [("qk", 1, 6), ("v", 5)],
            }
            s0_tail = [("qk", 1, 3), ("qk", 1, 7), ("v", 6), ("v", 7)]

            # prologue: just the two qk chunks head 0/1 need
            qk_tile(0, 0)
            qk_tile(0, 4)

            for s in range(TS):
                ntiles = 2 * s + 2
                # head 0's S tiles; interleave proj(s-1) tiles 0-1 as filler
                for jt in range(ntiles):
                    s_tile(s, 0, jt)
                    if s == 0:
                        for f in s0_block[0][2 * jt:2 * jt + 2]:
                            filler(f)
                    elif jt < 2:
                        proj_t4(s - 1, jt)
                mask_head(s, 0)
                if s == 0:  # PV(0) in block 1 reads V(2),V(3)
                    v_tile(2)
                    v_tile(3)
                # staggered: S(h) interleaved with PV(h-1); qk/V tiles of
                # span s+1 slot in as PE filler after each block
                for h in range(1, HPC):
                    ots = None
                    for jt in range(ntiles):
                        s_tile(s, h, jt)
                        ots = pv_chunks(s, h - 1, jt)
                    mask_head(s, h)
                    norm(s, h - 1, ots)
                    if s == 0:
                        for f in s0_block[h]:
                            filler(f)
                    elif s < TS - 1:
                        qk_tile(s + 1, h - 1)
                # tail: PV(7); proj(s-1) tiles 2-3, then qk/V filler that
                # also covers the next span's head-0 exp latency
                ots = None
                for jt in range(ntiles):
                    ots = pv_chunks(s, HPC - 1, jt)
                    if s > 0 and jt < 2:
                        proj_t4(s - 1, 2 + jt)
                norm(s, HPC - 1, ots)
                if s == 0:
                    for f in s0_tail:
                        filler(f)
                elif s < TS - 1:
                    qk_tile(s + 1, 7)
                    for t in range(4 * (s + 1), 4 * (s + 1) + 4):
                        v_tile(t)
            for t4 in range(4):
                proj_t4(TS - 1, t4)

    nc.compile()
    return nc


def _get_nc():
    global _nc_cache
    if _nc_cache is None:
        _nc_cache = _build()
    return _nc_cache


def kernel(x, w_attn, b_attn, w_proj, b_proj):
    x = np.asarray(x, dtype=np.float32)
    w_attn = np.asarray(w_attn, dtype=np.float32)
    b_attn = np.asarray(b_attn, dtype=np.float32)
    w_proj = np.asarray(w_proj, dtype=np.float32)
    b_proj = np.asarray(b_proj, dtype=np.float32)

    nc = _get_nc()

    ii = np.arange(128)
    mask1 = np.where(ii[None, :] <= ii[:, None], 1.0, 0.0).astype(np.float32).T
    maskB = np.broadcast_to(mask1[:, None, :], (128, 4, 128))

    def bf16(a):
        return np.ascontiguousarray(a.astype(ml_dtypes.bfloat16))

    in_maps = []
    for core in range(NCORES):
        b, g = core // 2, core % 2
        fs = slice(g * DH, (g + 1) * DH)
        wq = w_attn[:, fs] * 0.125  # fold 1/sqrt(HD)
        wk = w_attn[:, C + g * DH: C + (g + 1) * DH]
        wv = w_attn[:, 2 * C + g * DH: 2 * C + (g + 1) * DH]
        w2 = np.concatenate([wq, wk], axis=1)  # [C, 1024]
        wqk = w2.reshape(NC_CHUNKS, 128, 8, 128).transpose(2, 0, 1, 3)
        in_maps.append({
            "xT": bf16(x[b].T),
            "wqk": bf16(wqk),
            "wv": bf16(wv),
            "wp": bf16(w_proj[fs, :]),
            "maskB": bf16(np.ascontiguousarray(maskB)),
            "vones": np.ones((128, HPC), dtype=ml_dtypes.bfloat16),
            "ones64": np.ones((1, 64), dtype=ml_dtypes.bfloat16),
        })

    global LAST_RESULT
    res = run_bass_kernel_spmd(
        nc, in_maps, core_ids=list(range(NCORES)),
        trace=TRACE, **(TRACE_KW if TRACE else {}))
    LAST_RESULT = res

    corr = b_proj + b_attn[2 * C:3 * C] @ w_proj  # exact host-side bias fold
    out = np.empty((B, T, C), dtype=np.float32)
    for b in range(B):
        out[b] = res.results[2 * b]["out"] + res.results[2 * b + 1]["out"] + corr
    return out


# revision 21
# speedup vs baseline: 1.0432x; 1.0432x over previous
"""Causal self-attention (B=4, T=2048, C=1024, NH=16) on 8 trn2 NeuronCores.

Sharding: core = (batch b, head-half g); each core computes 8 heads of one
batch element and a partial projection output; host sums the two partials
per batch and folds in b_proj and the (softmax-row-sum==1) v-bias term.

b_attn's q/k components are assumed zero (spec fill: "zeros").

All matmul inputs are bf16 (1 cycle/row on the PE, cheap fast-weight-load
LDWEIGHTS, half the HBM traffic); accumulation is always f32 in PSUM.

Pipeline design (the previous version lost 2x+ to the PE HAM clock gate:
ScalarE exp was the per-head rate limiter, the PE micro-idled waiting on
it, and HAM throttled the PE clock to K=4/8 for ~380us of the attention
phase):
 - S^T matmuls write 2-key-chunk [128, 2, 512] PSUM tiles; ONE batched
   exp per tile ([128,1024]) amortizes ScalarE's 352-cycle fixed cost.
 - Heads are staggered: the PE stream interleaves S(h) tiles with
   PV(h-1) chunks, so the PE always has ready work while ScalarE exps
   head h (PV(h-1) inputs were finished last block).
 - Softmax normalization: DVE reciprocal_approx_fast on the ones-column
   rowsum (no Ln/Exp round trip, no activation-table switches), then a
   PE outer-product broadcast and one DVE multiply straight out of PSUM.
 - Projection of span s-1 is interleaved into the attention of span s,
   and projection results DMA to DRAM directly from PSUM.
"""

from contextlib import ExitStack

import ml_dtypes
import numpy as np

import concourse.bass as bass  # noqa: F401
import concourse.mybir as mybir
import concourse.tile as tile
from concourse import bacc
from concourse.bass_utils import run_bass_kernel_spmd

B, T, C, NH = 4, 2048, 1024, 16
HD = 64
NCORES = 8
HPC = NH // 2            # heads per core
DH = HPC * HD            # 512 per-core qkv feature width
TS = T // 512            # 4 query spans of 512
NT = T // 128            # 16 tiles of 128
NC_CHUNKS = C // 128     # 8 contraction chunks

F32 = mybir.dt.float32
F32R = mybir.dt.float32r
BF16 = mybir.dt.bfloat16
EXP = mybir.ActivationFunctionType.Exp

TRACE = False            # set by test.py for profiled runs
TRACE_KW = {}
LAST_RESULT = None

_nc_cache = None


def _build():
    nc = bacc.Bacc("TRN2", target_bir_lowering=False)

    xT_d = nc.dram_tensor("xT", [C, T], BF16, kind="ExternalInput")
    wqk_d = nc.dram_tensor("wqk", [8, NC_CHUNKS, 128, 128], BF16, kind="ExternalInput")
    wv_d = nc.dram_tensor("wv", [C, DH], BF16, kind="ExternalInput")
    wp_d = nc.dram_tensor("wp", [DH, C], BF16, kind="ExternalInput")
    maskB_d = nc.dram_tensor("maskB", [128, 4, 128], BF16, kind="ExternalInput")
    vones_d = nc.dram_tensor("vones", [128, HPC], BF16, kind="ExternalInput")
    ones64_d = nc.dram_tensor("ones64", [1, 64], BF16, kind="ExternalInput")
    out_d = nc.dram_tensor("out", [T, C], F32, kind="ExternalOutput")

    with tile.TileContext(nc) as tc, ExitStack() as ctx:
        const = ctx.enter_context(tc.tile_pool(name="const", bufs=1))
        persist = ctx.enter_context(tc.tile_pool(name="persist", bufs=1))

        maskB = const.tile([128, 4, 128], BF16)
        nc.sync.dma_start(maskB[:], maskB_d[:])
        ones64 = const.tile([1, 64], BF16)
        nc.sync.dma_start(ones64[:], ones64_d[:])

        # persistent SBUF: qT/kT bf16 [feat, T] (chunks 0-3 q, 4-7 k),
        # V bf16 [T-tile, head, 64+ones-col], wp bf16
        qk_sb = [persist.tile([128, T], BF16, tag=f"qk{i}", name=f"qk{i}")
                 for i in range(8)]
        v_sb = [persist.tile([128, HPC, 65], BF16, tag=f"v{i}", name=f"v{i}")
                for i in range(NT)]
        wp_sb = [persist.tile([128, C], BF16, tag=f"wp{i}", name=f"wp{i}")
                 for i in range(DH // 128)]
        for c in range(DH // 128):
            nc.sync.dma_start(wp_sb[c][:], wp_d[c * 128:(c + 1) * 128, :])
        for t in range(NT):
            nc.sync.dma_start(v_sb[t][:, :, 64], vones_d[:])

        # ---- merged QKV-projection + attention + projection --------------
        with tc.tile_pool(name="xT", bufs=1) as xpool, \
             tc.tile_pool(name="wqk", bufs=1) as wqkpool, \
             tc.tile_pool(name="wv", bufs=1) as wvpool, \
             tc.tile_pool(name="pt", bufs=1) as ptpool, \
             tc.tile_pool(name="yts", bufs=1) as ytspool, \
             tc.tile_pool(name="small", bufs=2) as small, \
             tc.tile_pool(name="outsb", bufs=2) as outsb, \
             tc.tile_pool(name="stps", bufs=2, space="PSUM") as stps, \
             tc.tile_pool(name="otps", bufs=2, space="PSUM") as otps, \
             tc.tile_pool(name="pprb", bufs=2, space="PSUM") as pprb:

            # DMA order matters: the first qk matmuls need xT span 0 and
            # the wqk chunks, so queue those ahead of the bulk of xT.
            xT_sb = [xpool.tile([128, T], BF16, tag=f"x{c}", name=f"x{c}")
                     for c in range(NC_CHUNKS)]
            for c in range(NC_CHUNKS):
                nc.sync.dma_start(
                    xT_sb[c][:, 0:512], xT_d[c * 128:(c + 1) * 128, 0:512])
            wts = []
            for ft in range(8):
                row = []
                for c in range(NC_CHUNKS):
                    wt = wqkpool.tile([128, 128], BF16, tag=f"wqk{ft}_{c}",
                                      name=f"wqk{ft}_{c}")
                    nc.sync.dma_start(wt[:], wqk_d[ft, c])
                    row.append(wt)
                wts.append(row)
            wv_sb = [wvpool.tile([128, DH], BF16, tag=f"wv{c}", name=f"wv{c}")
                     for c in range(NC_CHUNKS)]
            for c in range(NC_CHUNKS):
                nc.sync.dma_start(wv_sb[c][:], wv_d[c * 128:(c + 1) * 128, :])
            for ts in range(1, TS):
                for c in range(NC_CHUNKS):
                    nc.sync.dma_start(
                        xT_sb[c][:, ts * 512:(ts + 1) * 512],
                        xT_d[c * 128:(c + 1) * 128, ts * 512:(ts + 1) * 512])

            def qk_tile(ts, ft):
                ps = pprb.tile([128, 512], F32, tag="pp", name="qkp")
                for c in range(NC_CHUNKS):
                    nc.tensor.matmul(
                        ps[:], wts[ft][c][:],
                        xT_sb[c][:, ts * 512:(ts + 1) * 512],
                        start=(c == 0), stop=(c == NC_CHUNKS - 1))
                nc.vector.tensor_copy(
                    qk_sb[ft][:, ts * 512:(ts + 1) * 512], ps[:])

            def v_tile(t):
                vp = pprb.tile([128, 512], F32, tag="pp", name="vp")
                for c in range(NC_CHUNKS):
                    nc.tensor.matmul(
                        vp[:], xT_sb[c][:, t * 128:(t + 1) * 128],
                        wv_sb[c][:],
                        start=(c == 0), stop=(c == NC_CHUNKS - 1))
                nc.vector.tensor_copy(
                    v_sb[t][:, :, 0:64],
                    vp.rearrange("p (h d) -> p h d", h=HPC))

            # P~^T scratch, double buffered across heads:
            # [k-part, j-chunk, q-span], bf16
            pt = [ptpool.tile([128, NT, 512], BF16, tag=f"pt{i}",
                              name=f"pt{i}") for i in range(2)]
            # normalized attention outputs, double buffered across spans
            yts = [[ytspool.tile([128, 512], BF16, tag=f"yts{p}_{i}",
                                 name=f"yts{p}_{i}")
                    for i in range(DH // 128)] for p in range(2)]

            def s_tile(s, h, jt):
                """Two S^T chunk matmuls + masks + one batched exp."""
                qch, qrow = h // 2, 64 * (h % 2)
                qT = qk_sb[qch]
                kT = qk_sb[4 + qch]
                st = stps.tile([128, 2, 512], F32, tag="st", name="st")
                js = (2 * jt, 2 * jt + 1)
                for sl, j in enumerate(js):
                    qo = max(s * 512, j * 128)
                    w = (s + 1) * 512 - qo
                    nc.tensor.matmul(
                        st[:, sl, :w],
                        kT[qrow:qrow + 64, j * 128:(j + 1) * 128],
                        qT[qrow:qrow + 64, qo:qo + w],
                        start=True, stop=True)
                nc.scalar.activation(
                    pt[h % 2][:, js[0]:js[0] + 2, :], st[:, :, :], EXP)

            def mask_head(s, h):
                # multiplicative 0/1 causal mask on the 4 diagonal chunks'
                # first 128 columns, applied to pt AFTER exp: keeps the
                # Vector op off the exp critical path (PV reads pt a full
                # head-block later).
                nc.vector.tensor_tensor(
                    pt[h % 2][:, 4 * s:4 * s + 4, 0:128],
                    pt[h % 2][:, 4 * s:4 * s + 4, 0:128],
                    maskB[:], mybir.AluOpType.mult)

            def pv_chunks(s, h, jt):
                """Two P@V chunk matmuls for head h (exp'd last block)."""
                jmax = 4 * s + 3
                for j in (2 * jt, 2 * jt + 1):
                    qo = max(s * 512, j * 128)
                    w = (s + 1) * 512 - qo
                    rel = qo - s * 512
                    if j == 0:
                        ot = otps.tile([128, 512], F32, tag="ot", name="ot")
                        pv_chunks.ot = ot
                    ot = pv_chunks.ot
                    nc.tensor.matmul(
                        ot[0:65, rel:rel + w],
                        v_sb[j][:, h, :], pt[h % 2][:, j, :w],
                        start=(j == 0), stop=(j == jmax),
                        skip_group_check=True)
                return pv_chunks.ot

            def norm(s, h, ot):
                """yts(head block) = ot[0:64] * broadcast(1/rowsum)."""
                qch, qrow = h // 2, 64 * (h % 2)
                rsum = small.tile([1, 512], F32, tag="rsum", name="rsum")
                nc.vector.tensor_copy(rsum[:], ot[64:65, :])
                rinv = small.tile([1, 512], F32, tag="rinv", name="rinv")
                nc.vector.reciprocal_approx_fast(out=rinv[:], in_=rsum[:])
                rinvb = small.tile([1, 512], BF16, tag="rinvb", name="rinvb")
                nc.vector.tensor_copy(rinvb[:], rinv[:])
                rb = pprb.tile([128, 512], F32, tag="pp", name="rb")
                nc.tensor.matmul(rb[0:64, :], ones64[:], rinvb[:],
                                 start=True, stop=True)
                rbs = small.tile([64, 512], F32, tag="rbs", name="rbs")
                nc.vector.tensor_copy(rbs[:], rb[0:64, :])
                nc.vector.tensor_tensor(
                    yts[s % 2][qch][qrow:qrow + 64, :], ot[0:64, :],
                    rbs[:], mybir.AluOpType.mult)

            def proj_t4(sp, t4):
                """Project one 128-query tile of span sp; DMA from PSUM."""
                tt = sp * 4 + t4
                for n in range(2):
                    po = pprb.tile([128, 512], F32, tag="pp", name="pp")
                    for c in range(DH // 128):
                        nc.tensor.matmul(
                            po[:],
                            yts[sp % 2][c][:, t4 * 128:(t4 + 1) * 128],
                            wp_sb[c][:, n * 512:(n + 1) * 512],
                            start=(c == 0), stop=(c == DH // 128 - 1))
                    ob = outsb.tile([128, 512], F32, tag="ob", name="ob")
                    nc.vector.tensor_copy(ob[:], po[:])
                    nc.sync.dma_start(
                        out_d[tt * 128:(tt + 1) * 128,
                              n * 512:(n + 1) * 512], ob[:])

            # prologue: qk + V for span 0
            for ft in range(8):
                qk_tile(0, ft)
            for t in range(4):
                v_tile(t)

            for s in range(TS):
                ntiles = 2 * s + 2
                # head 0's S tiles; interleave proj(s-1) tiles 0-1 as filler
                for jt in range(ntiles):
                    s_tile(s, 0, jt)
                    if s > 0 and jt < 2:
                        proj_t4(s - 1, jt)
                mask_head(s, 0)
                # staggered: S(h) interleaved with PV(h-1); qk(span s+1)
                # projection tiles slot in as PE filler after each block
                for h in range(1, HPC):
                    ots = None
                    for jt in range(ntiles):
                        s_tile(s, h, jt)
                        ots = pv_chunks(s, h - 1, jt)
                    mask_head(s, h)
                    norm(s, h - 1, ots)
                    if s < TS - 1:
                        qk_tile(s + 1, h - 1)
                # tail: PV(7); proj(s-1) tiles 2-3, then qk/V filler that
                # also covers the next span's head-0 exp latency
                ots = None
                for jt in range(ntiles):
                    ots = pv_chunks(s, HPC - 1, jt)
                    if s > 0 and jt < 2:
                        proj_t4(s - 1, 2 + jt)
                norm(s, HPC - 1, ots)
                if s < TS - 1:
                    qk_tile(s + 1, 7)
                    for t in range(4 * (s + 1), 4 * (s + 1) + 4):
                        v_tile(t)
            for t4 in range(4):
                proj_t4(TS - 1, t4)

    nc.compile()
    return nc


def _get_nc():
    global _nc_cache
    if _nc_cache is None:
        _nc_cache = _build()
    return _nc_cache


def kernel(x, w_attn, b_attn, w_proj, b_proj):
    x = np.asarray(x, dtype=np.float32)
    w_attn = np.asarray(w_attn, dtype=np.float32)
    b_attn = np.asarray(b_attn, dtype=np.float32)
    w_proj = np.asarray(w_proj, dtype=np.float32)
    b_proj = np.asarray(b_proj, dtype=np.float32)

    nc = _get_nc()

    ii = np.arange(128)
    mask1 = np.where(ii[None, :] <= ii[:, None], 1.0, 0.0).astype(np.float32).T
    maskB = np.broadcast_to(mask1[:, None, :], (128, 4, 128))

    def bf16(a):
        return np.ascontiguousarray(a.astype(ml_dtypes.bfloat16))

    in_maps = []
    for core in range(NCORES):
        b, g = core // 2, core % 2
        fs = slice(g * DH, (g + 1) * DH)
        wq = w_attn[:, fs] * 0.125  # fold 1/sqrt(HD)
        wk = w_attn[:, C + g * DH: C + (g + 1) * DH]
        wv = w_attn[:, 2 * C + g * DH: 2 * C + (g + 1) * DH]
        w2 = np.concatenate([wq, wk], axis=1)  # [C, 1024]
        wqk = w2.reshape(NC_CHUNKS, 128, 8, 128).transpose(2, 0, 1, 3)
        in_maps.append({
            "xT": bf16(x[b].T),
            "wqk": bf16(wqk),
            "wv": bf16(wv),
            "wp": bf16(w_proj[fs, :]),
            "maskB": bf16(np.ascontiguousarray(maskB)),
            "vones": np.ones((128, HPC), dtype=ml_dtypes.bfloat16),
            "ones64": np.ones((1, 64), dtype=ml_dtypes.bfloat16),
        })

    global LAST_RESULT
    res = run_bass_kernel_spmd(
        nc, in_maps, core_ids=list(range(NCORES)),
        trace=TRACE, **(TRACE_KW if TRACE else {}))
    LAST_RESULT = res

    corr = b_proj + b_attn[2 * C:3 * C] @ w_proj  # exact host-side bias fold
    out = np.empty((B, T, C), dtype=np.float32)
    for b in range(B):
        out[b] = res.results[2 * b]["out"] + res.results[2 * b + 1]["out"] + corr
    return out
